# revision 11
# baseline (speedup 1.0000x reference)
"""Transformer block (pre-LN MHA + FFN) Trainium2 Bass kernel.

Data-parallel over 8 cores: core c handles batch b=c//2, sequence half c%2.
Each core computes LN1 + K/V over the batch's FULL 2048 rows (K/V duplicated
across the 2 cores sharing a batch), Q/attention/LN2/FFN over its own 1024
rows.  The core's own half is rolled to columns 0:1024 host-side (attention
is permutation-invariant over the k/v order), so the SPMD program is
identical on all cores.

Device layout: activations transposed [D-partitions, seq-free] throughout.
All big matmuls bf16 (fp32 PSUM accumulation). Softmax without
max-subtraction (scores ~ N(0,1)). Softmax denominators via an appended
ones-column on V; division deferred past attn@V via reciprocal + E-matrix
PE broadcast. LN stats via all-ones-lhsT matmuls (partition sums, free
row-broadcast); rstd = exp(-0.5*ln(var+eps)) to stay in one ACT table set.
LN gains/biases are folded into the projection weights host-side; the
second LN's g2/beta2 ride the FFN via W2a = [W2; diag(g2)], b2' = b2+beta2.

Self-contained: hardcodes shapes B=4, S=2048, D=1024, H=16, FF=4096.
"""

import numpy as np
import ml_dtypes

import concourse.bass as bass
import concourse.bacc as bacc
import concourse.tile as tile
from concourse import mybir

F32 = mybir.dt.float32
BF16 = mybir.dt.bfloat16
AF = mybir.ActivationFunctionType
OP = mybir.AluOpType

B, S, D, H, FF = 4, 2048, 1024, 16, 4096
HD = D // H          # 64
P = 128
DT = D // P          # 8  d-tiles
FT = FF // P         # 32 ff-tiles
FTA = FT + DT        # 40 ff-tiles with diag(g2) augmentation
KT = S // P          # 16 k-row tiles
SQ = S // 2          # 1024 own q columns per core
NQ = SQ // 512       # 2 q-chunks of 512
NS = S // 512        # 4 s-chunks of 512
EPS = 1e-5
NCORES = 8

_CACHE = {}


def _build_nc():
    nc = bacc.Bacc("TRN2", target_bir_lowering=False, debug=False,
                   num_devices=NCORES)

    xbf = nc.dram_tensor("xbf", [P, DT, S], BF16, kind="ExternalInput")
    xh = nc.dram_tensor("xh", [P, DT, SQ], F32, kind="ExternalInput")
    wq = nc.dram_tensor("wq", [P, DT, D], BF16, kind="ExternalInput")
    wk = nc.dram_tensor("wk", [P, DT, D], BF16, kind="ExternalInput")
    wv = nc.dram_tensor("wv", [P, DT, D], BF16, kind="ExternalInput")
    w1 = nc.dram_tensor("w1", [P, DT, FF], BF16, kind="ExternalInput")
    w2a = nc.dram_tensor("w2a", [P, FTA, D], BF16, kind="ExternalInput")
    bq = nc.dram_tensor("bq", [P, DT], F32, kind="ExternalInput")
    bk = nc.dram_tensor("bk", [P, DT], F32, kind="ExternalInput")
    bvb = nc.dram_tensor("bvb", [P, D], F32, kind="ExternalInput")
    b1 = nc.dram_tensor("b1", [P, FT], F32, kind="ExternalInput")
    b2 = nc.dram_tensor("b2", [P, DT], F32, kind="ExternalInput")
    emat = nc.dram_tensor("emat", [16, DT, P], F32, kind="ExternalInput")
    OUT = nc.dram_tensor("OUT", [P, DT, SQ], F32, kind="ExternalOutput")

    with tile.TileContext(nc) as tc:
        _emit(nc, tc, xbf, xh, wq, wk, wv, w1, w2a, bq, bk, bvb, b1, b2,
              emat, OUT)
    nc.compile()
    return nc


def _emit(nc, tc, xbf_d, xh_d, wq_d, wk_d, wv_d, w1_d, w2a_d, bq_d, bk_d,
          bvb_d, b1_d, b2_d, emat_d, OUT_d):
    pools = {}

    pool_objs = {}

    def open_pool(name, bufs, space="SBUF"):
        cm = tc.tile_pool(name=name, bufs=bufs, space=space)
        pools[name] = cm
        pool_objs[name] = cm.__enter__()
        return pool_objs[name]

    def close_pool(name):
        pools.pop(name).__exit__(None, None, None)

    # LIFO pool discipline: open in reverse order of close.
    p_const = open_pool("consts", 1)
    p_ps = open_pool("psg", 2, space="PSUM")
    p_fT = open_pool("fTp", 1)
    p_yb = open_pool("ybp", 1)
    p_att = open_pool("attn", 1)
    p_hT = open_pool("hTp", 1)

    # ---- constants ----
    ones_bf = p_const.tile([P, P], BF16, tag="ones")
    nc.vector.memset(ones_bf[:], 1.0)
    eps_t = p_const.tile([P, 1], F32, tag="eps")
    nc.vector.memset(eps_t[:], EPS)
    sb_bq = p_const.tile([P, DT], F32, tag="bq")
    nc.sync.dma_start(sb_bq[:], bq_d[:, :])
    sb_bk = p_const.tile([P, DT], F32, tag="bk")
    nc.sync.dma_start(sb_bk[:], bk_d[:, :])
    sb_bvb = p_const.tile([P, D], F32, tag="bvb")
    nc.sync.dma_start(sb_bvb[:], bvb_d[:, :])
    sb_b1 = p_const.tile([P, FT], F32, tag="b1")
    nc.sync.dma_start(sb_b1[:], b1_d[:, :])
    sb_b2 = p_const.tile([P, DT], F32, tag="b2")
    nc.sync.dma_start(sb_b2[:], b2_d[:, :])
    sb_emat = p_const.tile([16, DT, P], F32, tag="emat")
    nc.sync.dma_start(sb_emat[:], emat_d[:, :, :])

    hT = p_hT.tile([P, DT, S], BF16, tag="hT")
    attn_raw = p_att.tile([P, DT, SQ], BF16, tag="attn_raw")
    rgather = p_att.tile([16, SQ], BF16, tag="rgather")

    # =========================================================
    # Phase 1: LN1 over all S columns -> hT (bf16, [P, DT, S])
    # =========================================================
    p_ln1 = open_pool("ln1", 1)
    p_lt = open_pool("ln1tmp", 2)

    xbf = p_ln1.tile([P, DT, S], BF16, tag="xbf")
    nc.sync.dma_start(xbf[:], xbf_d[:, :, :])

    def emit_ln(src, ssl, dst, tmp_pool, sfx):
        """LN over partition-dim (all DT tiles) for one 512-col chunk."""
        xsq = tmp_pool.tile([P, DT, 512], BF16, tag="xsq" + sfx)
        nc.vector.tensor_mul(xsq[:], src[:, :, ssl], src[:, :, ssl])
        ps1 = p_ps.tile([P, 512], F32, tag="psg")
        ps2 = p_ps.tile([P, 512], F32, tag="psg")
        for dt in range(DT):
            nc.tensor.matmul(ps1[:], ones_bf[:], src[:, dt, ssl],
                             start=(dt == 0), stop=(dt == DT - 1))
        for dt in range(DT):
            nc.tensor.matmul(ps2[:], ones_bf[:], xsq[:, dt, :],
                             start=(dt == 0), stop=(dt == DT - 1))
        mu = tmp_pool.tile([P, 512], F32, tag="mu" + sfx)
        nc.vector.tensor_scalar_mul(mu[:], ps1[:], 1.0 / D)
        msq = tmp_pool.tile([P, 512], F32, tag="msq" + sfx)
        nc.vector.tensor_scalar_mul(msq[:], ps2[:], 1.0 / D)
        var = tmp_pool.tile([P, 512], F32, tag="var" + sfx)
        nc.vector.tensor_mul(var[:], mu[:], mu[:])
        nc.vector.tensor_sub(var[:], msq[:], var[:])
        lnv = tmp_pool.tile([P, 512], F32, tag="lnv" + sfx)
        nc.scalar.activation(lnv[:], var[:], AF.Ln, bias=eps_t[:], scale=1.0)
        rstd = tmp_pool.tile([P, 512], BF16, tag="rstd" + sfx)
        nc.scalar.activation(rstd[:], lnv[:], AF.Exp, bias=0.0, scale=-0.5)
        negmu = tmp_pool.tile([P, 512], F32, tag="negmu" + sfx)
        nc.vector.tensor_scalar_mul(negmu[:], mu[:], -1.0)
        nsb = tmp_pool.tile([P, 512], BF16, tag="nsb" + sfx)
        nc.vector.tensor_mul(nsb[:], negmu[:], rstd[:])
        tmpb = tmp_pool.tile([P, DT, 512], BF16, tag="tmpb" + sfx)
        nc.vector.tensor_tensor(
            tmpb[:], src[:, :, ssl],
            rstd[:, None, :].to_broadcast((P, DT, 512)), OP.mult)
        nc.vector.tensor_tensor(
            dst[:, :, ssl], tmpb[:],
            nsb[:, None, :].to_broadcast((P, DT, 512)), OP.add)

    for sc in range(NS):
        emit_ln(xbf, bass.ts(sc, 512), hT, p_lt, "a")

    close_pool("ln1tmp")
    close_pool("ln1")

    # =========================================================
    # Phase 2+3 interleaved: QKV projections and attention
    # =========================================================
    p_qkv = open_pool("qkvout", 1)
    qT = p_qkv.tile([P, DT, SQ], BF16, tag="qT")
    kT = p_qkv.tile([P, DT, S], BF16, tag="kT")
    vaug = p_qkv.tile([P, KT, H, HD + 1], BF16, tag="vaug")
    nc.vector.memset(vaug[:, :, :, HD:HD + 1], 1.0)

    p_w = open_pool("wslab", 2)
    p_sc = open_pool("scores", 2, space="PSUM")
    p_ap = open_pool("attps", 2, space="PSUM")
    p_ex = open_pool("expT", 3)
    p_st = open_pool("stage", 3)

    def own(qc):
        return bass.ts(qc, 512)

    def qkv_mtile(t):
        wq_s = p_w.tile([P, DT, P], BF16, tag="wq_s")
        nc.sync.dma_start(wq_s[:], wq_d[:, :, bass.ts(t, P)])
        wk_s = p_w.tile([P, DT, P], BF16, tag="wk_s")
        nc.sync.dma_start(wk_s[:], wk_d[:, :, bass.ts(t, P)])
        for qc in range(NQ):
            pq = p_ps.tile([P, 512], F32, tag="psg")
            for dt in range(DT):
                nc.tensor.matmul(pq[:], wq_s[:, dt, :], hT[:, dt, own(qc)],
                                 start=(dt == 0), stop=(dt == DT - 1))
            nc.vector.tensor_scalar(qT[:, t, own(qc)], pq[:],
                                    sb_bq[:, t:t + 1], None, OP.add)
        for sc in range(NS):
            pk = p_ps.tile([P, 512], F32, tag="psg")
            for dt in range(DT):
                nc.tensor.matmul(pk[:], wk_s[:, dt, :],
                                 hT[:, dt, bass.ts(sc, 512)],
                                 start=(dt == 0), stop=(dt == DT - 1))
            nc.vector.tensor_scalar(kT[:, t, bass.ts(sc, 512)], pk[:],
                                    sb_bk[:, t:t + 1], None, OP.add)

    def v_group(g):
        # v natural [k-rows, dout] for dout cols g*512..+512 (heads 8g..8g+7)
        wv_s = pool_objs["wvp"].tile([P, DT, 512], BF16, tag="wv_s")
        nc.sync.dma_start(wv_s[:], wv_d[:, :, bass.ts(g, 512)])
        for kt in range(KT):
            pv = p_ps.tile([P, 512], F32, tag="psg")
            for dt in range(DT):
                nc.tensor.matmul(pv[:], hT[:, dt, bass.ts(kt, P)],
                                 wv_s[:, dt, :],
                                 start=(dt == 0), stop=(dt == DT - 1))
            nc.vector.tensor_tensor(
                vaug[:, kt, 8 * g:8 * g + 8, 0:HD],
                pv[:].rearrange("p (h d) -> p h d", d=HD),
                sb_bvb[:, bass.ts(g, 512)].rearrange("p (h d) -> p h d", d=HD),
                OP.add)

    def attention_pair(t):
        # heads h0 = 2t (partitions 0:64 of dtile t), h1 = 2t+1 (64:128)
        for qc in range(NQ):
            aps = []
            for _i in range(2):
                ap_t = p_ap.tile([HD + 1, 512], F32, tag="attps",
                                 name=f"attps_{t}_{qc}_{_i}")
                aps.append(ap_t)
            for kt in range(KT):
                sc_ps = p_sc.tile([P, 2, 512], F32, tag="scps")
                for i in range(2):
                    pb = 64 * i
                    nc.tensor.matmul(
                        sc_ps[:, i, :],
                        kT[pb:pb + 64, t, bass.ts(kt, P)],
                        qT[pb:pb + 64, t, own(qc)],
                        start=True, stop=True)
                ex = p_ex.tile([P, 2, 512], BF16, tag="expT")
                nc.scalar.activation(ex[:], sc_ps[:], AF.Exp,
                                     bias=0.0, scale=0.125)
                for i in range(2):
                    nc.tensor.matmul(aps[i][:],
                                     vaug[:, kt, 2 * t + i, :],
                                     ex[:, i, :],
                                     start=(kt == 0), stop=(kt == KT - 1))
            for i in range(2):
                h = 2 * t + i
                st = p_st.tile([HD + 1, 512], BF16, tag="stage")
                nc.vector.tensor_copy(st[:], aps[i][:])
                nc.sync.dma_start(
                    attn_raw[64 * i:64 * i + 64, t, own(qc)], st[0:HD, :])
                nc.sync.dma_start(rgather[h:h + 1, own(qc)],
                                  st[HD:HD + 1, :])

    for t in range(4):
        qkv_mtile(t)
    open_pool("wvp", 1)
    v_group(0)
    close_pool("wvp")
    for t in range(4):
        attention_pair(t)
    for t in range(4, DT):
        qkv_mtile(t)
    open_pool("wvp", 1)
    v_group(1)
    close_pool("wvp")
    for t in range(4, DT):
        attention_pair(t)

    close_pool("stage")
    close_pool("expT")
    close_pool("attps")
    close_pool("scores")
    close_pool("wslab")
    close_pool("qkvout")
    close_pool("hTp")

    # =========================================================
    # Phase 4: y = attn_raw * R_b + xh   (bf16 y)
    # =========================================================
    p_y = open_pool("yph", 1)
    p_yt = open_pool("ytmp", 3)

    rpad = p_y.tile([16, SQ], F32, tag="rpad")
    nc.vector.reciprocal(rpad[:], rgather[:])
    sb_xh = p_y.tile([P, DT, SQ], F32, tag="xh")
    nc.sync.dma_start(sb_xh[:], xh_d[:, :, :])
    ybf = p_yb.tile([P, DT, SQ], BF16, tag="ybf")

    for dt in range(DT):
        for qc in range(NQ):
            rb = p_ps.tile([P, 512], F32, tag="psg")
            nc.tensor.matmul(rb[:], sb_emat[:, dt, :], rpad[:, own(qc)],
                             start=True, stop=True)
            t1 = p_yt.tile([P, 512], F32, tag="t1")
            nc.vector.tensor_mul(t1[:], attn_raw[:, dt, own(qc)], rb[:])
            nc.vector.tensor_add(ybf[:, dt, own(qc)], t1[:],
                                 sb_xh[:, dt, own(qc)])

    close_pool("ytmp")
    close_pool("yph")
    close_pool("attn")

    # =========================================================
    # Phase 5: LN2 over own SQ columns -> fT (bf16)
    # =========================================================
    p_l2 = open_pool("ln2tmp", 3)
    fT = p_fT.tile([P, DT, SQ], BF16, tag="fT")

    for sc in range(NQ):
        emit_ln(ybf, bass.ts(sc, 512), fT, p_l2, "b")

    close_pool("ln2tmp")
    close_pool("ybp")

    # =========================================================
    # Phase 6: FFN.  out = relu(fT@W1'+b1') @ W2 + fT*g2 + b2'
    # (W2a = [W2; diag(g2)] augmentation; b2' = b2 + beta2)
    # =========================================================
    p_f = open_pool("ffn", 1)
    p_fr = open_pool("relu", 1)
    p_fw = open_pool("w2slab", 2)
    p_fo = open_pool("fout", 3)

    sb_w1 = p_f.tile([P, DT, FF], BF16, tag="w1")
    nc.sync.dma_start(sb_w1[:], w1_d[:, :, :])

    for qc in range(NQ):
        relu = p_fr.tile([P, FT, 512], BF16, tag="relu")
        for ft in range(FT):
            pf = p_ps.tile([P, 512], F32, tag="psg")
            for dt in range(DT):
                nc.tensor.matmul(pf[:], sb_w1[:, dt, bass.ts(ft, P)],
                                 fT[:, dt, own(qc)],
                                 start=(dt == 0), stop=(dt == DT - 1))
            nc.vector.tensor_scalar(relu[:, ft, :], pf[:],
                                    sb_b1[:, ft:ft + 1], 0.0,
                                    OP.add, OP.max)
        for mt in range(DT):
            slab = p_fw.tile([P, FTA, P], BF16, tag="w2slab")
            nc.sync.dma_start(slab[:], w2a_d[:, :, bass.ts(mt, P)])
            po = p_ps.tile([P, 512], F32, tag="psg")
            for ft in range(FTA):
                rhs = relu[:, ft, :] if ft < FT else fT[:, ft - FT, own(qc)]
                nc.tensor.matmul(po[:], slab[:, ft, :], rhs,
                                 start=(ft == 0), stop=(ft == FTA - 1))
            ot = p_fo.tile([P, 512], F32, tag="ot")
            nc.vector.tensor_scalar(ot[:], po[:], sb_b2[:, mt:mt + 1], None,
                                    OP.add)
            nc.sync.dma_start(OUT_d[:, mt, own(qc)], ot[:])

    close_pool("fout")
    close_pool("w2slab")
    close_pool("relu")
    close_pool("ffn")
    close_pool("fTp")
    close_pool("psg")
    close_pool("consts")


def _prep_shared(inputs):
    """Host-side weight preprocessing (shared across cores)."""
    f32 = np.float32
    g1 = np.asarray(inputs["g1"], f32)
    beta1 = np.asarray(inputs["beta1"], f32)
    g2 = np.asarray(inputs["g2"], f32)
    beta2 = np.asarray(inputs["beta2"], f32)
    Wq = np.asarray(inputs["Wq"], f32)
    Wk = np.asarray(inputs["Wk"], f32)
    Wv = np.asarray(inputs["Wv"], f32)
    W1 = np.asarray(inputs["W1"], f32)
    W2 = np.asarray(inputs["W2"], f32)

    def fold(Wm, bm):
        Wp = Wm * g1[:, None]
        bp = np.asarray(inputs[bm], f32) + beta1 @ Wm
        return Wp, bp

    Wqp, bqp = fold(Wq, "bq")
    Wkp, bkp = fold(Wk, "bk")
    Wvp, bvp = fold(Wv, "bv")
    W1p = W1 * g2[:, None]
    b1p = np.asarray(inputs["b1"], f32) + beta2 @ W1
    W2a = np.concatenate([W2, np.diag(g2)], axis=0)          # [FF+D, D]
    b2p = np.asarray(inputs["b2"], f32) + beta2

    bf = ml_dtypes.bfloat16

    def wtile(Wm, ntile):
        return np.ascontiguousarray(
            Wm.reshape(ntile, P, Wm.shape[1]).transpose(1, 0, 2)).astype(bf)

    def btile(bv, ntile):
        return np.ascontiguousarray(bv.reshape(ntile, P).T).astype(f32)

    E = np.zeros((16, DT, P), f32)
    for t in range(DT):
        for m in range(P):
            E[2 * t + m // HD, t, m] = 1.0

    return {
        "wq": wtile(Wqp, DT), "wk": wtile(Wkp, DT), "wv": wtile(Wvp, DT),
        "w1": wtile(W1p, DT), "w2a": wtile(W2a, FTA),
        "bq": btile(bqp, DT), "bk": btile(bkp, DT),
        "bvb": np.ascontiguousarray(np.broadcast_to(bvp, (P, D))).astype(f32),
        "b1": btile(b1p, FT), "b2": btile(b2p, DT),
        "emat": E,
    }


def _per_core_inputs(inputs, shared):
    x = np.asarray(inputs["x"], np.float32)
    maps = []
    for c in range(NCORES):
        b, hf = c // 2, c % 2
        xTn = x[b].T.reshape(DT, P, S).transpose(1, 0, 2)
        if hf == 1:
            # roll so this core's own half is always columns 0:SQ
            xTn = np.concatenate([xTn[:, :, SQ:], xTn[:, :, :SQ]], axis=2)
        xTn = np.ascontiguousarray(xTn)
        m = dict(shared)
        m["xbf"] = xTn.astype(ml_dtypes.bfloat16)
        m["xh"] = np.ascontiguousarray(xTn[:, :, :SQ])
        maps.append(m)
    return maps


def _get_sharded():
    """Build (once) the nc + jitted shard_map executable."""
    if "sharded" in _CACHE:
        return _CACHE["sharded"]

    import jax
    from jax.sharding import Mesh, PartitionSpec
    from jax.experimental.shard_map import shard_map
    from concourse import bass2jax
    from concourse import mybir as _mybir

    bass2jax.install_neuronx_cc_hook()
    nc = _build_nc()

    partition_name = (nc.partition_id_tensor.name
                      if nc.partition_id_tensor else None)
    in_names, out_names, out_avals, zero_shapes = [], [], [], []
    for alloc in nc.m.functions[0].allocations:
        if not isinstance(alloc, _mybir.MemoryLocationSet):
            continue
        name = alloc.memorylocations[0].name
        if alloc.kind == "ExternalInput":
            if name != partition_name:
                in_names.append(name)
        elif alloc.kind == "ExternalOutput":
            shape = tuple(alloc.tensor_shape)
            dtype = _mybir.dt.np(alloc.dtype)
            out_names.append(name)
            out_avals.append(jax.core.ShapedArray(shape, dtype))
            zero_shapes.append((shape, dtype))
    n_params = len(in_names)
    all_names = in_names + out_names
    if partition_name is not None:
        all_names = all_names + [partition_name]
    donate = tuple(range(n_params, n_params + len(out_names)))

    def _body(*args):
        operands = list(args)
        if partition_name is not None:
            operands.append(bass2jax.partition_id_tensor())
        outs = bass2jax._bass_exec_p.bind(
            *operands,
            out_avals=tuple(out_avals),
            in_names=tuple(all_names),
            out_names=tuple(out_names),
            lowering_input_output_aliases=(),
            sim_require_finite=True,
            sim_require_nnan=True,
            nc=nc,
        )
        return tuple(outs)

    devices = jax.devices()[:NCORES]
    mesh = Mesh(np.asarray(devices), ("core",))
    nin = n_params + len(out_names)
    sharded = jax.jit(
        shard_map(_body, mesh=mesh,
                  in_specs=(PartitionSpec("core"),) * nin,
                  out_specs=(PartitionSpec("core"),) * len(out_names),
                  check_rep=False),
        donate_argnums=donate, keep_unused=True)

    _CACHE["sharded"] = (nc, sharded, in_names, out_names, out_avals,
                         zero_shapes)
    return _CACHE["sharded"]


def _concat_inputs(in_maps):
    _, _, in_names, _, _, zero_shapes = _get_sharded()
    concat_in = [
        np.concatenate([np.asarray(in_maps[c][n]) for c in range(NCORES)],
                       axis=0)
        for n in in_names
    ]
    concat_zeros = [
        np.zeros((NCORES * s[0], *s[1:]), d) for (s, d) in zero_shapes
    ]
    return concat_in, concat_zeros


def _run(in_maps):
    nc, fn, in_names, out_names, out_avals, zero_shapes = _get_sharded()
    concat_in, concat_zeros = _concat_inputs(in_maps)
    outs = fn(*concat_in, *concat_zeros)
    res = []
    for c in range(NCORES):
        res.append({
            name: np.asarray(outs[i]).reshape(NCORES, *out_avals[i].shape)[c]
            for i, name in enumerate(out_names)
        })
    return res


def kernel(**inputs):
    shared = _prep_shared(inputs)
    in_maps = _per_core_inputs(inputs, shared)
    res = _run(in_maps)
    out = np.empty((B, S, D), np.float32)
    for c in range(NCORES):
        b, hf = c // 2, c % 2
        o = res[c]["OUT"]                       # [P, DT, SQ]
        out[b, hf * SQ:(hf + 1) * SQ, :] = o.transpose(2, 1, 0).reshape(SQ, D)
    return out
